# revision 4
# baseline (speedup 1.0000x reference)
"""Trainium2 Bass kernel for causal multi-head attention (B=2, T=2048, C=1024, H=16).

Reference semantics (see problem):
    qp = q @ Wq + bq ; kp = k @ Wk + bk ; vp = kp @ Wv + bv   (V from projected K)
    S  = (qh @ khT) / sqrt(D), causal mask, A = softmax(S)
    ctx = A @ vh ; out = ctx @ Wo + bo
Returns (out, attention_weights).

Sharding: 8 cores; core c handles batch b = c//4 and 4 heads h0 = 4*(c%4).
Each core gets transposed activations qT/kT [C, T] (host-transposed), the
head-sliced weights, and the host-folded Wkv = Wk @ Wv[:, slice] so vp is
computed directly from k without materializing full kp. Scores scale 1/8 is
folded into Wq/bq on the host. Partial output projections are summed on the
host (the all-reduce step of the sharding strategy); bo added there too.

Device per core: project qpT/kpT [256, T] (features on partitions) and
vp [T, 256]; per head/q-tile compute S via K=64 matmuls, add additive causal
mask on the diagonal block, exp on ScalarE (no max subtraction needed: |S|<~6)
with fused row-sum accumulation, normalize, DMA A out; PE-transpose the A
tiles to feed the PV matmul producing ctxT [64, T] per head; final out
projection from ctxT with Wo slice.
"""
import math
import numpy as np
from contextlib import ExitStack

import concourse.mybir as mybir
import concourse.tile as tile
from concourse import bacc
from concourse.bass_utils import run_bass_kernel_spmd
from concourse.masks import make_identity, make_causal_mask

B, T, C, H = 2, 2048, 1024, 16
D = C // H            # 64
NCORES = 8
HPC = 4               # heads per core
HD = HPC * D          # 256
CK = C // 128         # 8 contraction chunks
QT = T // 128         # 16 q tiles
F32 = mybir.dt.float32

ACT = mybir.ActivationFunctionType


def _build():
    nc = bacc.Bacc("TRN2", target_bir_lowering=False, debug=False,
                   num_devices=NCORES)
    qT = nc.dram_tensor("qT", [C, T], F32, kind="ExternalInput").ap()
    kT = nc.dram_tensor("kT", [C, T], F32, kind="ExternalInput").ap()
    wq = nc.dram_tensor("wq", [C, HD], F32, kind="ExternalInput").ap()
    wk = nc.dram_tensor("wk", [C, HD], F32, kind="ExternalInput").ap()
    wkv = nc.dram_tensor("wkv", [C, HD], F32, kind="ExternalInput").ap()
    wo = nc.dram_tensor("wo", [HD, C], F32, kind="ExternalInput").ap()
    bq = nc.dram_tensor("bq", [2, 128, 1], F32, kind="ExternalInput").ap()
    bk = nc.dram_tensor("bk", [2, 128, 1], F32, kind="ExternalInput").ap()
    bkv = nc.dram_tensor("bkv", [1, HD], F32, kind="ExternalInput").ap()
    attnw = nc.dram_tensor("attnw", [HPC, T, T], F32, kind="ExternalOutput").ap()
    outp = nc.dram_tensor("outp", [T, C], F32, kind="ExternalOutput").ap()

    with tile.TileContext(nc) as tc, ExitStack() as ctx:
        consts = ctx.enter_context(tc.tile_pool(name="consts", bufs=1))
        wpool = ctx.enter_context(tc.tile_pool(name="wpool", bufs=1))
        big = ctx.enter_context(tc.tile_pool(name="big", bufs=1))
        acts = ctx.enter_context(tc.tile_pool(name="acts", bufs=1))
        apool = ctx.enter_context(tc.tile_pool(name="apool", bufs=2))
        atp = ctx.enter_context(tc.tile_pool(name="atp", bufs=4))
        small = ctx.enter_context(tc.tile_pool(name="small", bufs=4))
        opool = ctx.enter_context(tc.tile_pool(name="opool", bufs=2))
        ps = ctx.enter_context(tc.tile_pool(name="ps", bufs=4, space="PSUM"))
        psc = ctx.enter_context(tc.tile_pool(name="psc", bufs=2, space="PSUM"))

        # constants
        ident_t = consts.tile([128, 128], F32, tag="ident")
        make_identity(nc, ident_t[:])
        diag_t = consts.tile([128, 128], F32, tag="diag")
        make_causal_mask(nc, diag_t[:], mask_val=-1e30)
        zero_t = consts.tile([128, 512], F32, tag="zero")
        nc.gpsimd.memset(zero_t[:], 0.0)
        ones_t = consts.tile([1, 128], F32, tag="ones")
        nc.gpsimd.memset(ones_t[:], 1.0)

        # weights
        wq_t = wpool.tile([128, CK, HD], F32, tag="wq")
        wk_t = wpool.tile([128, CK, HD], F32, tag="wk")
        wkv_t = wpool.tile([128, CK, HD], F32, tag="wkv")
        wo_t = wpool.tile([128, 2, C], F32, tag="wo")
        nc.sync.dma_start(wq_t[:], wq.rearrange("(ck p) d -> p ck d", p=128))
        nc.sync.dma_start(wk_t[:], wk.rearrange("(ck p) d -> p ck d", p=128))
        nc.sync.dma_start(wkv_t[:], wkv.rearrange("(ck p) d -> p ck d", p=128))
        nc.sync.dma_start(wo_t[:], wo.rearrange("(dc p) c -> p dc c", p=128))
        bq_t = consts.tile([128, 2, 1], F32, tag="bq")
        bk_t = consts.tile([128, 2, 1], F32, tag="bk")
        bkv_t = consts.tile([1, HD], F32, tag="bkv")
        nc.sync.dma_start(bq_t[:], bq.rearrange("m p one -> p m one"))
        nc.sync.dma_start(bk_t[:], bk.rearrange("m p one -> p m one"))
        nc.sync.dma_start(bkv_t[:], bkv[:])

        # persistent activations
        qpT_t = [acts.tile([128, T], F32, tag=f"qpT{m}", name=f"qpT{m}") for m in range(2)]
        kpT_t = [acts.tile([128, T], F32, tag=f"kpT{m}", name=f"kpT{m}") for m in range(2)]
        vp_t = acts.tile([128, QT, HD], F32, tag="vp")
        ctxT_t = [acts.tile([128, T], F32, tag=f"ctxT{m}", name=f"ctxT{m}") for m in range(2)]

        # ---- Phase 1: projections (stream qT/kT in halves of T) ----
        qT_r = qT.rearrange("(ck p) t -> p ck t", p=128)
        kT_r = kT.rearrange("(ck p) t -> p ck t", p=128)
        TH = T // 2
        for th in range(2):
            xh = big.tile([128, CK, TH], F32, tag="xT")
            nc.sync.dma_start(xh[:], qT_r[:, :, th * TH:(th + 1) * TH])
            for m in range(2):
                for tc512 in range(TH // 512):
                    p = ps.tile([128, 512], F32, tag="pproj")
                    for ck in range(CK):
                        nc.tensor.matmul(
                            p[:], wq_t[:, ck, m * 128:(m + 1) * 128],
                            xh[:, ck, tc512 * 512:(tc512 + 1) * 512],
                            start=(ck == 0), stop=(ck == CK - 1))
                    nc.scalar.activation(
                        qpT_t[m][:, th * TH + tc512 * 512:th * TH + (tc512 + 1) * 512],
                        p[:], ACT.Identity, bias=bq_t[:, m, :])
        for th in range(2):
            xh = big.tile([128, CK, TH], F32, tag="xT")
            nc.sync.dma_start(xh[:], kT_r[:, :, th * TH:(th + 1) * TH])
            for m in range(2):
                for tc512 in range(TH // 512):
                    p = ps.tile([128, 512], F32, tag="pproj")
                    for ck in range(CK):
                        nc.tensor.matmul(
                            p[:], wk_t[:, ck, m * 128:(m + 1) * 128],
                            xh[:, ck, tc512 * 512:(tc512 + 1) * 512],
                            start=(ck == 0), stop=(ck == CK - 1))
                    nc.scalar.activation(
                        kpT_t[m][:, th * TH + tc512 * 512:th * TH + (tc512 + 1) * 512],
                        p[:], ACT.Identity, bias=bk_t[:, m, :])
            # vp for the t-tiles in this half: vp[t,:] = k @ Wkv + bkv
            for ti in range(th * (QT // 2), (th + 1) * (QT // 2)):
                tl = ti * 128 - th * TH
                p = ps.tile([128, 512], F32, tag="pproj")
                for ck in range(CK):
                    nc.tensor.matmul(
                        p[:, :HD], xh[:, ck, tl:tl + 128], wkv_t[:, ck, :],
                        start=(ck == 0), stop=False)
                nc.tensor.matmul(p[:, :HD], ones_t[:], bkv_t[:],
                                 start=False, stop=True)
                nc.scalar.activation(vp_t[:, ti, :], p[:, :HD], ACT.Copy)

        # ---- Phase 2: attention, processed in head pairs ----
        # Heads 2hp and 2hp+1 sit at partition offsets 0/64 of qpT_t[hp]; their
        # K=64 score matmuls target disjoint PE row-groups, so issuing them
        # back-to-back lets the PE run both concurrently (row tiling).
        ctx_sb = acts.tile([128, QT, HD], F32, tag="ctx", name="ctx_sb")
        for hp in range(2):
            for i in range(QT):
                win = (i + 1) * 128
                nch = (win + 511) // 512
                dk = (i * 128) // 512
                a_s = [apool.tile([128, T], F32, tag=f"astrip{g}", name=f"astrip{g}")
                       for g in range(2)]
                rs = [small.tile([128, 4], F32, tag=f"rs{g}", name=f"rs{g}")
                      for g in range(2)]
                pps = {}
                for kc in range(nch):
                    n = min(512, win - kc * 512)
                    for g in range(2):
                        off = 64 * g
                        p = ps.tile([128, 512], F32, tag="pproj", name="pscore")
                        pps[g] = p
                        nc.tensor.matmul(
                            p[:, :n],
                            qpT_t[hp][off:off + 64, i * 128:(i + 1) * 128],
                            kpT_t[hp][off:off + 64, kc * 512:kc * 512 + n],
                            start=True, stop=True)
                    for g in range(2):
                        p = pps[g]
                        if kc == dk:
                            pos = i * 128 - dk * 512
                            nc.vector.tensor_tensor(
                                out=p[:, pos:pos + 128], in0=p[:, pos:pos + 128],
                                in1=diag_t[:], op=mybir.AluOpType.add)
                        nc.scalar.activation(a_s[g][:, kc * 512:kc * 512 + n],
                                             p[:, :n], ACT.Exp,
                                             accum_out=rs[g][:, kc:kc + 1])
                for g in range(2):
                    h = 2 * hp + g
                    rtot = small.tile([128, 1], F32, tag="rtot", name="rtot")
                    nc.vector.tensor_reduce(rtot[:], rs[g][:, :nch],
                                            axis=mybir.AxisListType.X,
                                            op=mybir.AluOpType.add)
                    rcp = small.tile([128, 1], F32, tag="rcp", name="rcp")
                    nc.vector.reciprocal(rcp[:], rtot[:])
                    nc.vector.tensor_scalar_mul(a_s[g][:, :win], a_s[g][:, :win],
                                                rcp[:])
                    nc.sync.dma_start(attnw[h, i * 128:(i + 1) * 128, 0:win],
                                      a_s[g][:, :win])
                    z = win
                    while z < T:
                        n = min(512, T - z)
                        nc.sync.dma_start(
                            attnw[h, i * 128:(i + 1) * 128, z:z + n],
                            zero_t[:, :n])
                        z += n
                    # PV: ctx[q, d] += A^T-tile.T @ vp tile (moving dim 64)
                    pc = psc.tile([128, 64], F32, tag="pctx", name="pctx")
                    for kc in range(i + 1):
                        pt = psc.tile([128, 128], F32, tag="ptr", name="ptr")
                        nc.tensor.transpose(pt[:],
                                            a_s[g][:, kc * 128:(kc + 1) * 128],
                                            ident_t[:])
                        at = atp.tile([128, 128], F32, tag="at", name="at")
                        nc.any.tensor_copy(at[:], pt[:])
                        nc.tensor.matmul(pc[:], at[:],
                                         vp_t[:, kc, h * 64:(h + 1) * 64],
                                         start=(kc == 0), stop=(kc == i))
                    nc.any.tensor_copy(ctx_sb[:, i, h * 64:(h + 1) * 64], pc[:])

        # ctx [q, d] -> ctxT [d, q] for the output projection
        for dc in range(2):
            for i in range(QT):
                pt = psc.tile([128, 128], F32, tag="ptr", name="ptr")
                nc.tensor.transpose(
                    pt[:], ctx_sb[:, i, dc * 128:(dc + 1) * 128], ident_t[:])
                nc.any.tensor_copy(ctxT_t[dc][:, i * 128:(i + 1) * 128], pt[:])

        # ---- Phase 3: output projection (partial; host sums across cores) ----
        for i in range(QT):
            o_sb = opool.tile([128, C], F32, tag="osb")
            for n2 in range(2):
                p = ps.tile([128, 512], F32, tag="pproj", name="pout")
                for dc in range(2):
                    nc.tensor.matmul(p[:],
                                     ctxT_t[dc][:, i * 128:(i + 1) * 128],
                                     wo_t[:, dc, n2 * 512:(n2 + 1) * 512],
                                     start=(dc == 0), stop=(dc == 1))
                nc.any.tensor_copy(o_sb[:, n2 * 512:(n2 + 1) * 512], p[:])
            nc.sync.dma_start(outp[i * 128:(i + 1) * 128, :], o_sb[:])

    nc.compile()
    return nc


_cached = {}


def _get_prog():
    if "nc" not in _cached:
        _cached["nc"] = _build()
    return _cached["nc"]


def _prep_inputs(q, k, Wq, bq, Wk, bk, Wv, bv, Wo, bo, mask):
    """Build the 8 per-core input maps (host-side sharding)."""
    q = np.asarray(q, np.float32)
    k = np.asarray(k, np.float32)
    Wq = np.asarray(Wq, np.float64)
    Wk = np.asarray(Wk, np.float64)
    Wv = np.asarray(Wv, np.float64)
    Wo = np.asarray(Wo, np.float64)
    bq = np.asarray(bq, np.float64)
    bk = np.asarray(bk, np.float64)
    bv = np.asarray(bv, np.float64)
    scale = 1.0 / math.sqrt(D)
    in_maps = []
    for core in range(NCORES):
        b = core // 4
        h0 = HPC * (core % 4)
        sl = slice(h0 * D, (h0 + HPC) * D)
        qTb = np.ascontiguousarray(q[b].T)
        kTb = np.ascontiguousarray(k[b].T)
        wq_s = np.ascontiguousarray(Wq[:, sl] * scale).astype(np.float32)
        wk_s = np.ascontiguousarray(Wk[:, sl]).astype(np.float32)
        wkv_s = np.ascontiguousarray(Wk @ Wv[:, sl]).astype(np.float32)
        wo_s = np.ascontiguousarray(Wo[sl, :]).astype(np.float32)
        bq_s = (bq[sl] * scale).astype(np.float32).reshape(2, 128, 1)
        bk_s = bk[sl].astype(np.float32).reshape(2, 128, 1)
        bkv_s = (bk @ Wv[:, sl] + bv[sl]).astype(np.float32).reshape(1, HD)
        in_maps.append({
            "qT": qTb, "kT": kTb, "wq": wq_s, "wk": wk_s, "wkv": wkv_s,
            "wo": wo_s, "bq": bq_s, "bk": bk_s, "bkv": bkv_s,
        })
    return in_maps


def _run(inputs, trace=False, trace_kwargs=None):
    nc = _get_prog()
    in_maps = _prep_inputs(**inputs)
    res = run_bass_kernel_spmd(nc, in_maps, list(range(NCORES)), trace=trace,
                               **(trace_kwargs or {}))
    bo = np.asarray(inputs["bo"], np.float64)
    out = np.empty((B, T, C), np.float32)
    attnw = np.empty((B, H, T, T), np.float32)
    for b in range(B):
        acc = np.zeros((T, C), np.float64)
        for cc in range(4):
            core = b * 4 + cc
            acc += res.results[core]["outp"]
            attnw[b, cc * HPC:(cc + 1) * HPC] = res.results[core]["attnw"]
        out[b] = (acc + bo).astype(np.float32)
    return (out, attnw), res


def kernel(**inputs):
    (out, attnw), _ = _run(inputs, trace=False)
    return out, attnw


# revision 6
# speedup vs baseline: 1.8337x; 1.8337x over previous
"""Trainium2 Bass kernel for causal multi-head attention (B=2, T=2048, C=1024, H=16).

Reference semantics:
    qp = q @ Wq + bq ; kp = k @ Wk + bk ; vp = kp @ Wv + bv   (V from projected K)
    S  = (qh @ khT) / sqrt(D), causal mask, A = softmax(S)
    ctx = A @ vh ; out = ctx @ Wo + bo
Returns (out, attention_weights).

Sharding: 8 cores; core c handles batch b = c//4 and 4 heads h0 = 4*(c%4).
Each core gets transposed activations qT/kT [C, T] (host-transposed), head-
sliced weights, and host-folded Wkv = Wk @ Wv[:, slice] so vp comes straight
from k. The scores scale 1/sqrt(D) is folded into Wq/bq. Partial output
projections are summed on the host (the all-reduce of the sharding strategy);
bo added there. The upper-triangle zeros of attention_weights are filled on
the host, the device only writes the causal part.

Matmul dtype: float32r (PE reads fp32 operands rounded to ~12-bit mantissa,
runs 4x faster than true fp32). Set RDT = F32 below for full fp32.
"""
import math
import numpy as np
from contextlib import ExitStack

import concourse.mybir as mybir
import concourse.tile as tile
from concourse import bacc
from concourse.bass_utils import run_bass_kernel_spmd
from concourse.masks import make_identity, make_causal_mask

B, T, C, H = 2, 2048, 1024, 16
D = C // H            # 64
NCORES = 8
HPC = 4               # heads per core
HD = HPC * D          # 256
CK = C // 128         # 8 contraction chunks
QT = T // 128         # 16 q tiles
F32 = mybir.dt.float32
RDT = mybir.dt.float32r   # matmul operand dtype

ACT = mybir.ActivationFunctionType


def _build():
    nc = bacc.Bacc("TRN2", target_bir_lowering=False, debug=False,
                   num_devices=NCORES)
    qT = nc.dram_tensor("qT", [C, T], F32, kind="ExternalInput").ap()
    kT = nc.dram_tensor("kT", [C, T], F32, kind="ExternalInput").ap()
    wq = nc.dram_tensor("wq", [C, HD], F32, kind="ExternalInput").ap()
    wk = nc.dram_tensor("wk", [C, HD], F32, kind="ExternalInput").ap()
    wkv = nc.dram_tensor("wkv", [C, HD], F32, kind="ExternalInput").ap()
    wo = nc.dram_tensor("wo", [HD, C], F32, kind="ExternalInput").ap()
    bq = nc.dram_tensor("bq", [2, 128, 1], F32, kind="ExternalInput").ap()
    bk = nc.dram_tensor("bk", [2, 128, 1], F32, kind="ExternalInput").ap()
    bkv = nc.dram_tensor("bkv", [1, HD], F32, kind="ExternalInput").ap()
    attnw = nc.dram_tensor("attnw", [HPC, T, T], F32, kind="ExternalOutput").ap()
    outp = nc.dram_tensor("outp", [T, C], F32, kind="ExternalOutput").ap()

    with tile.TileContext(nc) as tc, ExitStack() as ctx:
        consts = ctx.enter_context(tc.tile_pool(name="consts", bufs=1))
        wpool = ctx.enter_context(tc.tile_pool(name="wpool", bufs=1))
        big = ctx.enter_context(tc.tile_pool(name="big", bufs=1))
        acts = ctx.enter_context(tc.tile_pool(name="acts", bufs=1))
        apool = ctx.enter_context(tc.tile_pool(name="apool", bufs=5))
        atp = ctx.enter_context(tc.tile_pool(name="atp", bufs=3))
        small = ctx.enter_context(tc.tile_pool(name="small", bufs=4))
        opool = ctx.enter_context(tc.tile_pool(name="opool", bufs=2))
        ps = ctx.enter_context(tc.tile_pool(name="ps", bufs=4, space="PSUM"))
        psc = ctx.enter_context(tc.tile_pool(name="psc", bufs=2, space="PSUM"))

        # constants
        ident_t = consts.tile([128, 128], F32, tag="ident", name="ident_t")
        make_identity(nc, ident_t[:])
        diag_t = consts.tile([128, 128], F32, tag="diag", name="diag_t")
        make_causal_mask(nc, diag_t[:], mask_val=-1e30)
        ones_f = consts.tile([1, 128], F32, tag="ones_f", name="ones_f")
        nc.gpsimd.memset(ones_f[:], 1.0)
        ones_t = consts.tile([1, 128], RDT, tag="ones", name="ones_t")
        nc.vector.tensor_copy(ones_t[:], ones_f[:])
        zerof_t = consts.tile([128, 128], F32, tag="zerof", name="zerof_t")
        nc.gpsimd.memset(zerof_t[:], 0.0)
        zeror_t = consts.tile([128, 128], RDT, tag="zeror", name="zeror_t")
        nc.vector.tensor_copy(zeror_t[:], zerof_t[:])

        # weights (DMA'd straight into matmul operand dtype)
        wq_t = wpool.tile([128, CK, HD], RDT, tag="wq", name="wq_t")
        wk_t = wpool.tile([128, CK, HD], RDT, tag="wk", name="wk_t")
        wkv_t = wpool.tile([128, CK, HD], RDT, tag="wkv", name="wkv_t")
        wo_t = wpool.tile([128, 2, C], RDT, tag="wo", name="wo_t")
        nc.sync.dma_start(wq_t[:], wq.rearrange("(ck p) d -> p ck d", p=128).bitcast(RDT))
        nc.sync.dma_start(wk_t[:], wk.rearrange("(ck p) d -> p ck d", p=128).bitcast(RDT))
        nc.sync.dma_start(wkv_t[:], wkv.rearrange("(ck p) d -> p ck d", p=128).bitcast(RDT))
        nc.sync.dma_start(wo_t[:], wo.rearrange("(dc p) c -> p dc c", p=128).bitcast(RDT))
        bq_t = consts.tile([128, 2, 1], F32, tag="bq", name="bq_t")
        bk_t = consts.tile([128, 2, 1], F32, tag="bk", name="bk_t")
        bkv_t = consts.tile([1, HD], RDT, tag="bkv", name="bkv_t")
        nc.sync.dma_start(bq_t[:], bq.rearrange("m p one -> p m one"))
        nc.sync.dma_start(bk_t[:], bk.rearrange("m p one -> p m one"))
        nc.sync.dma_start(bkv_t[:], bkv.bitcast(RDT))

        # persistent activations (matmul operands -> RDT)
        qpT_t = [acts.tile([128, T], RDT, tag=f"qpT{m}", name=f"qpT{m}")
                 for m in range(2)]
        kpT_t = [acts.tile([128, T], RDT, tag=f"kpT{m}", name=f"kpT{m}")
                 for m in range(2)]
        vp_t = acts.tile([128, QT, HD], RDT, tag="vp", name="vp_t")
        ctxT_t = [acts.tile([128, T], RDT, tag=f"ctxT{m}", name=f"ctxT{m}")
                  for m in range(2)]

        # ---- Phase 1: projections (stream qT/kT in halves of T) ----
        qT_r = qT.rearrange("(ck p) t -> p ck t", p=128).bitcast(RDT)
        kT_r = kT.rearrange("(ck p) t -> p ck t", p=128).bitcast(RDT)
        TH = T // 2
        for th in range(2):
            xh = big.tile([128, CK, TH], RDT, tag="xT", name="xh")
            nc.sync.dma_start(xh[:], qT_r[:, :, th * TH:(th + 1) * TH])
            for m in range(2):
                for tc512 in range(TH // 512):
                    p = ps.tile([128, 512], F32, tag="pproj", name="pproj")
                    for ck in range(CK):
                        nc.tensor.matmul(
                            p[:], wq_t[:, ck, m * 128:(m + 1) * 128],
                            xh[:, ck, tc512 * 512:(tc512 + 1) * 512],
                            start=(ck == 0), stop=(ck == CK - 1))
                    nc.scalar.activation(
                        qpT_t[m][:, th * TH + tc512 * 512:th * TH + (tc512 + 1) * 512],
                        p[:], ACT.Identity, bias=bq_t[:, m, :])
        for th in range(2):
            xh = big.tile([128, CK, TH], RDT, tag="xT", name="xh")
            nc.sync.dma_start(xh[:], kT_r[:, :, th * TH:(th + 1) * TH])
            for m in range(2):
                for tc512 in range(TH // 512):
                    p = ps.tile([128, 512], F32, tag="pproj", name="pproj")
                    for ck in range(CK):
                        nc.tensor.matmul(
                            p[:], wk_t[:, ck, m * 128:(m + 1) * 128],
                            xh[:, ck, tc512 * 512:(tc512 + 1) * 512],
                            start=(ck == 0), stop=(ck == CK - 1))
                    nc.scalar.activation(
                        kpT_t[m][:, th * TH + tc512 * 512:th * TH + (tc512 + 1) * 512],
                        p[:], ACT.Identity, bias=bk_t[:, m, :])
            # vp for this half: vp[t,:] = k @ Wkv + bkv (K=1 ones trick for bias)
            for ti in range(th * (QT // 2), (th + 1) * (QT // 2)):
                tl = ti * 128 - th * TH
                p = ps.tile([128, 512], F32, tag="pproj", name="pproj")
                for ck in range(CK):
                    nc.tensor.matmul(
                        p[:, :HD], xh[:, ck, tl:tl + 128], wkv_t[:, ck, :],
                        start=(ck == 0), stop=False)
                nc.tensor.matmul(p[:, :HD], ones_t[:], bkv_t[:],
                                 start=False, stop=True)
                nc.scalar.activation(vp_t[:, ti, :], p[:, :HD], ACT.Copy)

        # ---- Phase 2: attention per head, q-windows of 512 for the PV stage ----
        for h in range(HPC):
            ht = h // 2
            off = 64 * (h % 2)
            for j in range(4):
                a_s = {}
                for i in range(4 * j, 4 * j + 4):
                    win = (i + 1) * 128
                    nch = (win + 511) // 512
                    dk = (i * 128) // 512
                    a_s[i] = apool.tile([128, T], F32, tag="astrip",
                                        name="astrip")
                    rs = small.tile([128, 4], F32, tag="rs", name="rs")
                    for kc in range(nch):
                        n = min(512, win - kc * 512)
                        p = ps.tile([128, 512], F32, tag="pproj", name="pscore")
                        nc.tensor.matmul(
                            p[:, :n],
                            qpT_t[ht][off:off + 64, i * 128:(i + 1) * 128],
                            kpT_t[ht][off:off + 64, kc * 512:kc * 512 + n],
                            start=True, stop=True)
                        if kc == dk:
                            pos = i * 128 - dk * 512
                            nc.vector.tensor_tensor(
                                out=p[:, pos:pos + 128],
                                in0=p[:, pos:pos + 128],
                                in1=diag_t[:], op=mybir.AluOpType.add)
                        nc.scalar.activation(a_s[i][:, kc * 512:kc * 512 + n],
                                             p[:, :n], ACT.Exp,
                                             accum_out=rs[:, kc:kc + 1])
                    rtot = small.tile([128, 1], F32, tag="rtot", name="rtot")
                    nc.vector.tensor_reduce(rtot[:], rs[:, :nch],
                                            axis=mybir.AxisListType.X,
                                            op=mybir.AluOpType.add)
                    rcp = small.tile([128, 1], F32, tag="rcp", name="rcp")
                    nc.vector.reciprocal(rcp[:], rtot[:])
                    nc.vector.tensor_scalar_mul(a_s[i][:, :win], a_s[i][:, :win],
                                                rcp[:])
                    nc.sync.dma_start(attnw[h, i * 128:(i + 1) * 128, 0:win],
                                      a_s[i][:, :win])
                # PV over this q-window: ctxT[d, 512] accumulated over kc
                pc = psc.tile([64, 512], F32, tag="pctx", name="pctx")
                nkc = 4 * j + 4
                for kc in range(nkc):
                    at = atp.tile([128, 512], RDT, tag="at", name="at")
                    for q4 in range(4):
                        i = 4 * j + q4
                        if i < kc:
                            nc.vector.tensor_copy(at[:, q4 * 128:(q4 + 1) * 128],
                                                  zeror_t[:])
                        else:
                            pt = psc.tile([128, 128], F32, tag="ptr", name="ptr")
                            nc.tensor.transpose(
                                pt[:], a_s[i][:, kc * 128:(kc + 1) * 128],
                                ident_t[:])
                            nc.any.tensor_copy(at[:, q4 * 128:(q4 + 1) * 128],
                                               pt[:])
                    nc.tensor.matmul(pc[:], vp_t[:, kc, h * 64:(h + 1) * 64],
                                     at[:], start=(kc == 0), stop=(kc == nkc - 1))
                nc.any.tensor_copy(
                    ctxT_t[ht][off:off + 64, j * 512:(j + 1) * 512], pc[:])

        # ---- Phase 3: output projection (partial; host sums across cores) ----
        for i in range(QT):
            o_sb = opool.tile([128, C], F32, tag="osb", name="o_sb")
            for n2 in range(2):
                p = ps.tile([128, 512], F32, tag="pproj", name="pout")
                for dc in range(2):
                    nc.tensor.matmul(p[:],
                                     ctxT_t[dc][:, i * 128:(i + 1) * 128],
                                     wo_t[:, dc, n2 * 512:(n2 + 1) * 512],
                                     start=(dc == 0), stop=(dc == 1))
                nc.any.tensor_copy(o_sb[:, n2 * 512:(n2 + 1) * 512], p[:])
            nc.sync.dma_start(outp[i * 128:(i + 1) * 128, :], o_sb[:])

    nc.compile()
    return nc


_cached = {}


def _get_prog():
    if "nc" not in _cached:
        _cached["nc"] = _build()
    return _cached["nc"]


def _prep_inputs(q, k, Wq, bq, Wk, bk, Wv, bv, Wo, bo, mask):
    """Build the 8 per-core input maps (host-side sharding)."""
    q = np.asarray(q, np.float32)
    k = np.asarray(k, np.float32)
    Wq = np.asarray(Wq, np.float64)
    Wk = np.asarray(Wk, np.float64)
    Wv = np.asarray(Wv, np.float64)
    Wo = np.asarray(Wo, np.float64)
    bq = np.asarray(bq, np.float64)
    bk = np.asarray(bk, np.float64)
    bv = np.asarray(bv, np.float64)
    scale = 1.0 / math.sqrt(D)
    in_maps = []
    for core in range(NCORES):
        b = core // 4
        h0 = HPC * (core % 4)
        sl = slice(h0 * D, (h0 + HPC) * D)
        in_maps.append({
            "qT": np.ascontiguousarray(q[b].T),
            "kT": np.ascontiguousarray(k[b].T),
            "wq": np.ascontiguousarray(Wq[:, sl] * scale).astype(np.float32),
            "wk": np.ascontiguousarray(Wk[:, sl]).astype(np.float32),
            "wkv": np.ascontiguousarray(Wk @ Wv[:, sl]).astype(np.float32),
            "wo": np.ascontiguousarray(Wo[sl, :]).astype(np.float32),
            "bq": (bq[sl] * scale).astype(np.float32).reshape(2, 128, 1),
            "bk": bk[sl].astype(np.float32).reshape(2, 128, 1),
            "bkv": (bk @ Wv[:, sl] + bv[sl]).astype(np.float32).reshape(1, HD),
        })
    return in_maps


def _run(inputs, trace=False, trace_kwargs=None):
    nc = _get_prog()
    in_maps = _prep_inputs(**inputs)
    res = run_bass_kernel_spmd(nc, in_maps, list(range(NCORES)), trace=trace,
                               **(trace_kwargs or {}))
    bo = np.asarray(inputs["bo"], np.float64)
    out = np.empty((B, T, C), np.float32)
    attnw = np.zeros((B, H, T, T), np.float32)
    for b in range(B):
        acc = np.zeros((T, C), np.float64)
        for cc in range(4):
            core = b * 4 + cc
            acc += res.results[core]["outp"]
            dev = res.results[core]["attnw"]
            dst = attnw[b, cc * HPC:(cc + 1) * HPC]
            # device wrote only the causal part; upper triangle stays zero
            for i in range(QT):
                win = (i + 1) * 128
                dst[:, i * 128:(i + 1) * 128, :win] = \
                    dev[:, i * 128:(i + 1) * 128, :win]
        out[b] = (acc + bo).astype(np.float32)
    return (out, attnw), res


def kernel(**inputs):
    (out, attnw), _ = _run(inputs, trace=False)
    return out, attnw
